# revision 43
# baseline (speedup 1.0000x reference)
"""BinaryLinear (8192x4096 @ 4096x4096 binarized) on 8 TRN2 NeuronCores.

Strategy (tensor-parallel, column sharding per out_features):
  - Shard W/alpha/b along out_features: each core gets 512 output channels.
  - Replicate x (host pre-transposed to [in_f, n_rows] so the contraction
    dim lands on SBUF partitions without any device-side transpose).
  - Weights are binarized on the HOST: bw = sign(W) * alpha. With per-channel
    alpha, +-alpha is exact in bf16, and for the fp8 K-range +-alpha is cast
    to e4m3 (exact for alpha=1). No device-side weight prep at all, so the
    first matmul can start as soon as the first W k-tile + x tile land.
  - Hybrid precision over the contraction dim K (the x quantization is the
    only real error source; binary weights are exact in every dtype):
      * K[0:KF8)   in fp8-e4m3 with MatmulPerfMode.DoubleRow (2 K-rows per
        PE cycle -> 2x matmul throughput). PE upcasts e4m3 exactly (e6m3),
        products +-x8 are exact in e10m10, accumulation fp32.
      * K[KF8:4096) in bf16 (1 K-row per cycle).
    Measured on the fixed problem inputs: rel_err ~ 2.65e-2 * sqrt(KF8/4096)
    (+ ~1.6e-3 bf16 floor in quadrature). KF8=1792 -> 1.761e-2 < 2e-2 gate,
    deterministic (host fp64 simulation of the quantization predicts the
    device error to 7 digits).
  - Per core matmul: x tile is the stationary operand ([128,128] bf16 or
    [128,2,128] fp8), binarized-W k-tile the moving operand ([128,512] bf16
    or [128,2,512] fp8 = 1024 moving rows), accumulating [128,512] PSUM.
  - Host gathers the 8 [8192, 512] shards with a concatenate on axis 1.
"""

import os
import sys

sys.path.insert(0, "/opt/trn_rl_repo")

import numpy as np
import ml_dtypes

from concourse import bacc, bass, mybir
import concourse.tile as tile
from concourse.bass_utils import run_bass_kernel_spmd

N_ROWS = 8192
IN_F = 4096
OUT_F = 4096
N_CORES = 8
O_SHARD = OUT_F // N_CORES  # 512

P = 128
# fp8 K columns (mult of 256). Measured on the fixed problem inputs (gate
# 2e-2): 1792 -> rel_err 1.761e-2, 2048 -> 1.881e-2; deterministic (host-
# predictable to 7 digits). 1792 is the default: 2048 is ~10us faster on a
# cool chip (365us vs 377us) but its higher sustained power gets stuck in
# the P0 2.0GHz downclock when the chip is pre-heated by earlier runs
# (430us measured); 1792 degrades only to ~388us in that state and carries
# 2x the correctness margin.
KF8 = int(os.environ.get("KF8", "1792"))
INTERLEAVE = os.environ.get("ILV", "1") == "1"
WARM = int(os.environ.get("WARM", "12"))   # warm-up matmuls
GATE6 = int(os.environ.get("GATE6", "4"))  # lead gate at sched[GATE6/6]
XBUFS = int(os.environ.get("XBUFS", "12"))

VARIANT = "hyb"


def build_nc_hyb(
    n_rows=N_ROWS,
    in_f=IN_F,
    o_shard=O_SHARD,
    kf8=KF8,
    n_chunk=512,
    x_bufs=XBUFS,
    interleave=INTERLEAVE,
):
    """Hybrid fp8-DoubleRow + bf16 per-core graph (same program, SPMD)."""
    f32 = mybir.dt.float32
    bf16 = mybir.dt.bfloat16
    f8 = mybir.dt.float8e4

    kbf = in_f - kf8
    assert kf8 % 256 == 0 and kbf % P == 0
    assert n_rows % n_chunk == 0 and n_chunk % P == 0
    o_mm = min(512, o_shard)  # one PSUM bank per [128, o_mm] fp32 chunk
    OCH = o_shard // o_mm
    assert OCH * o_mm == o_shard
    KO8 = kf8 // 256
    KOB = kbf // P
    NCH = n_rows // n_chunk
    NS = n_chunk // P
    psum_bufs = 2 if NS * OCH * 2 <= 8 else 1

    nc = bacc.Bacc("TRN2", target_bir_lowering=False)

    # logical k = ko*256 + two*128 + p on the fp8 side (both operands use the
    # same mapping, so the contraction is consistent), ko*128 + p on bf16.
    if KO8 > 0:
        xT8 = nc.declare_dram_parameter("xT8", [kf8, n_rows], f8, isOutput=False)
        WT8 = nc.declare_dram_parameter("WT8", [kf8, o_shard], f8, isOutput=False)
        xT8_t = xT8[:].rearrange("(ko two p) n -> ko p two n", p=P, two=2)
        WT8_t = WT8[:].rearrange("(ko two p) o -> ko p two o", p=P, two=2)
    if KOB > 0:
        xTb = nc.declare_dram_parameter("xTb", [kbf, n_rows], bf16, isOutput=False)
        WTb = nc.declare_dram_parameter("WTb", [kbf, o_shard], bf16, isOutput=False)
        xTb_t = xTb[:].rearrange("(ko p) n -> ko p n", p=P)
        WTb_t = WTb[:].rearrange("(ko p) o -> ko p o", p=P)
    b_rep = nc.declare_dram_parameter("b_rep", [P, o_shard], f32, isOutput=False)
    out = nc.declare_dram_parameter("out", [n_rows, o_shard], f32, isOutput=True)

    # schedule of k-steps; spread the fp8 DoubleRow steps evenly among the
    # bf16 steps to keep instantaneous PE power flat: a dense run of 2x-MAC
    # DoubleRow matmuls trips the P0 power downclock, slowing the WHOLE
    # kernel to ~2.0 GHz (measured: blocked 445us vs interleaved 377us).
    if interleave and KO8 > 0 and KOB > 0:
        sched = []
        i8 = ib = 0
        for s in range(KO8 + KOB):
            if i8 * KOB <= ib * KO8 and i8 < KO8:
                sched.append(("f8", i8))
                i8 += 1
            else:
                sched.append(("bf", ib))
                ib += 1
    else:
        sched = [("f8", ko) for ko in range(KO8)] + [
            ("bf", ko) for ko in range(KOB)
        ]

    with tile.TileContext(nc) as tc:
        with (
            tc.tile_pool(name="consts", bufs=1) as consts,
            tc.tile_pool(name="xp", bufs=x_bufs) as xp,
            tc.tile_pool(name="xlast", bufs=1) as xlast,
            tc.tile_pool(name="outp", bufs=4) as outp,
            tc.tile_pool(name="psum", bufs=psum_bufs, space="PSUM") as psump,
        ):
            # Weight loads ride the scalar+gpsimd HWDGE queues so the x-tile
            # stream (sync queue) never waits behind them, issued in schedule
            # order so chunk 0's first k-steps have their weights first.
            if KO8 > 0:
                W8 = consts.tile([P, KO8, 2, o_shard], f8)
            if KOB > 0:
                Wb = consts.tile([P, KOB, o_shard], bf16)
            for si, (kind, ko) in enumerate(sched):
                if kind == "f8":
                    nc.scalar.dma_start(out=W8[:, ko], in_=WT8_t[ko])
                else:
                    nc.scalar.dma_start(out=Wb[:, ko], in_=WTb_t[ko])
            b_sb = consts.tile([P, o_shard], f32)
            nc.scalar.dma_start(out=b_sb[:], in_=b_rep[:])

            # all x tiles on the sync queue: measured lower PE-side sem-wait
            # exposure than splitting the fp8 stream onto gpsimd (gpsimd's
            # DMA trigger issue is slower and lags the PE's consumption).
            def x_tile(kind, ko, nsl, pool, suf="", bufs=None):
                if kind == "f8":
                    t = pool.tile(
                        [P, 2, n_chunk], f8, tag="x8" + suf, name="x8", bufs=bufs
                    )
                    nc.sync.dma_start(out=t[:], in_=xT8_t[ko, :, :, nsl])
                else:
                    t = pool.tile(
                        [P, n_chunk], bf16, tag="xb" + suf, name="xb", bufs=bufs
                    )
                    nc.sync.dma_start(out=t[:], in_=xTb_t[ko, :, nsl])
                return t

            def mm(psum, x_t, kind, ko, ns, start, stop, och=0):
                osl = slice(och * o_mm, (och + 1) * o_mm)
                if kind == "f8":
                    nc.tensor.matmul(
                        psum[:],
                        x_t[:, :, ns * P : (ns + 1) * P],
                        W8[:, ko, :, osl],
                        start=start,
                        stop=stop,
                        perf_mode=mybir.MatmulPerfMode.DoubleRow,
                    )
                else:
                    nc.tensor.matmul(
                        psum[:],
                        x_t[:, ns * P : (ns + 1) * P],
                        Wb[:, ko, osl],
                        start=start,
                        stop=stop,
                    )

            def drain(psums_ns, row0):
                o_sb = outp.tile([P, o_shard], f32, tag="o")
                for och in range(OCH):
                    osl = slice(och * o_mm, (och + 1) * o_mm)
                    nc.vector.tensor_tensor(
                        o_sb[:, osl], psums_ns[och][:], b_sb[:, osl],
                        mybir.AluOpType.add,
                    )
                nc.scalar.dma_start(out=out[row0 : row0 + P, :], in_=o_sb[:])

            # Chunk 0's x tiles are all prefetched up front so the sync queue
            # banks a full chunk of lead over the PE before real compute
            # starts. All 8 cores pull HBM in lockstep, so a lead lost at
            # kernel start never rebuilds — every x handoff then exposes a
            # ~200ns semaphore wait for the rest of the run.
            tiles0 = {}
            nsl0 = slice(0, n_chunk)
            for kind, ko in sched:
                tiles0[(kind, ko)] = x_tile(
                    kind, ko, nsl0, xlast, suf="0",
                    bufs=KO8 if kind == "f8" else KOB,
                )

            # Meanwhile, warm the PE's HAM clock gate (cold = 1.2 GHz for the
            # first ~3.4us of activity) with dummy matmuls on zeroed SBUF,
            # then gate on a mid-chunk tile's DMA so the PE starts real work
            # only once the lead is banked (the gate matmul computes into the
            # dead warm psum).
            # NOTE on pacing: a fully dense matmul stream at this fp8
            # fraction trips the P0 power downclock (whole kernel drops to
            # ~2.0 GHz, +20%). The occasional ~200ns x-handoff waits of the
            # just-in-time stream keep sustained power below the threshold,
            # so do NOT over-provision the x lead (warm=44/gate=5/6/bufs=20
            # measured 443us vs 365us for this config).
            warm_sb = consts.tile([P, o_shard], bf16)
            nc.vector.memset(warm_sb[:], 0.0)
            warm_ps = psump.tile([P, o_shard], f32, tag="ps0", name="warm")
            for _ in range(WARM):
                nc.tensor.matmul(
                    warm_ps[:],
                    warm_sb[:, :P],
                    warm_sb[:],
                    start=True,
                    stop=True,
                )
            gate_kind, gate_ko = sched[(GATE6 * len(sched)) // 6]
            mm(warm_ps, tiles0[(gate_kind, gate_ko)], gate_kind, gate_ko, 0,
               True, True)

            last = len(sched) - 1
            for nch in range(NCH - 1):
                nsl = slice(nch * n_chunk, (nch + 1) * n_chunk)
                psums = [
                    psump.tile([P, o_shard], f32, tag=f"ps{ns}", name=f"ps{ns}")
                    for ns in range(NS)
                ]
                for si, (kind, ko) in enumerate(sched):
                    if nch == 0:
                        x_t = tiles0[(kind, ko)]
                    else:
                        x_t = x_tile(kind, ko, nsl, xp)
                    for ns in range(NS):
                        mm(psums[ns], x_t, kind, ko, ns, si == 0, si == last)
                for ns in range(NS):
                    drain(psums[ns], nch * n_chunk + ns * P)

            # Last chunk runs ns-outer / k-inner over prefetched x tiles so
            # the four psum drains overlap compute instead of serializing
            # after the final matmul (tail was ~13us with the k-outer order).
            nch = NCH - 1
            nsl = slice(nch * n_chunk, (nch + 1) * n_chunk)
            tiles = {}
            for kind, ko in sched:
                tiles[(kind, ko)] = x_tile(
                    kind, ko, nsl, xlast, suf="L",
                    bufs=KO8 if kind == "f8" else KOB,
                )
            for ns in range(NS):
                psum = psump.tile([P, o_shard], f32, tag=f"ps{ns}", name=f"ps{ns}")
                for si, (kind, ko) in enumerate(sched):
                    mm(psum, tiles[(kind, ko)], kind, ko, ns, si == 0, si == last)
                drain(psum, nch * n_chunk + ns * P)
    nc.compile()
    return nc


def make_in_maps_hyb(x, W, alpha, b, n_cores=N_CORES, kf8=KF8):
    """Host-side shard + binarize + quantize (no device weight prep)."""
    o_shard = W.shape[0] // n_cores
    xT = np.ascontiguousarray(x.T)
    xT8 = np.ascontiguousarray(xT[:kf8]).astype(ml_dtypes.float8_e4m3)
    xTb = np.ascontiguousarray(xT[kf8:]).astype(ml_dtypes.bfloat16)
    bwT = np.ascontiguousarray(
        (np.where(W >= 0, 1.0, -1.0).astype(np.float32) * alpha).T
    )
    in_maps = []
    for c in range(n_cores):
        sl = slice(c * o_shard, (c + 1) * o_shard)
        m = {
            "b_rep": np.ascontiguousarray(
                np.broadcast_to(b[sl].reshape(1, -1), (P, o_shard)),
                dtype=np.float32,
            ),
        }
        if kf8 > 0:
            m["xT8"] = xT8
            m["WT8"] = np.ascontiguousarray(bwT[:kf8, sl]).astype(
                ml_dtypes.float8_e4m3
            )
        if kf8 < xT.shape[0]:
            m["xTb"] = xTb
            m["WTb"] = np.ascontiguousarray(bwT[kf8:, sl]).astype(
                ml_dtypes.bfloat16
            )
        in_maps.append(m)
    return in_maps


_NC_CACHE = {}


def kernel(x, W, alpha, b, trace=False, variant=VARIANT):
    x = np.asarray(x, dtype=np.float32)
    W = np.asarray(W, dtype=np.float32)
    alpha = np.asarray(alpha, dtype=np.float32)
    b = np.asarray(b, dtype=np.float32)

    n_rows, in_f = x.shape
    out_f = W.shape[0]
    o_shard = out_f // N_CORES

    key = (n_rows, in_f, variant)
    if key not in _NC_CACHE:
        _NC_CACHE[key] = build_nc_hyb(
            n_rows=n_rows, in_f=in_f, o_shard=o_shard
        )
    nc = _NC_CACHE[key]

    in_maps = make_in_maps_hyb(x, W, alpha, b)
    res = None
    for attempt in range(3):
        try:
            res = run_bass_kernel_spmd(
                nc, in_maps, core_ids=list(range(N_CORES)), trace=trace
            )
            break
        except Exception:
            # Two failure modes seen in practice: (a) the trace path needs
            # antenv.axon_hooks + artifact upload, which some containers
            # lack; (b) transient NRT_EXEC_UNIT_UNRECOVERABLE device state.
            # Retry untraced unless the caller explicitly asked for a trace.
            if trace:
                raise
            os.environ["BASS_NEVER_TRACE"] = "1"
            trace = False
            if attempt == 2:
                raise
    full = np.empty((n_rows, out_f), dtype=np.float32)
    for c in range(N_CORES):
        full[:, c * o_shard : (c + 1) * o_shard] = np.asarray(
            res.results[c]["out"]
        )
    if trace:
        return full, res
    return full


if __name__ == "__main__":
    # small-scale CoreSim numeric check
    from concourse.bass_interp import CoreSim

    rng = np.random.default_rng(0)
    n_rows, in_f, o_shard, kf8 = 256, 1024, 256, 512
    x = rng.standard_normal((n_rows, in_f)).astype(np.float32)
    W = rng.standard_normal((o_shard, in_f)).astype(np.float32) * 0.02
    alpha = np.ones((o_shard, 1), np.float32)
    b = (rng.standard_normal(o_shard) * 0.01).astype(np.float32)

    nc = build_nc_hyb(
        n_rows=n_rows, in_f=in_f, o_shard=o_shard, kf8=kf8, n_chunk=256
    )
    print("build ok")
    sim = CoreSim(nc)
    xT = np.ascontiguousarray(x.T)
    xT8 = xT[:kf8].astype(ml_dtypes.float8_e4m3)
    xTb = xT[kf8:].astype(ml_dtypes.bfloat16)
    bwT = np.ascontiguousarray((np.where(W >= 0, 1.0, -1.0) * alpha).T)
    sim.tensor("xT8")[:] = xT8
    sim.tensor("xTb")[:] = xTb
    sim.tensor("WT8")[:] = bwT[:kf8].astype(ml_dtypes.float8_e4m3)
    sim.tensor("WTb")[:] = bwT[kf8:].astype(ml_dtypes.bfloat16)
    sim.tensor("b_rep")[:] = np.broadcast_to(b.reshape(1, -1), (P, o_shard))
    sim.simulate(check_with_hw=False)
    got = np.array(sim.tensor("out"))
    want = (
        np.concatenate(
            [
                xT8.astype(np.float32).T,
                xTb.astype(np.float32).T,
            ],
            axis=1,
        )
        @ bwT
        + b
    )
    rel = np.linalg.norm(got - want) / np.linalg.norm(want)
    print("sim rel err vs quantized-exact:", rel)
    full = x @ (np.where(W >= 0, 1.0, -1.0) * alpha).T + b
    print(
        "sim rel err vs exact:",
        np.linalg.norm(got - full) / np.linalg.norm(full),
    )
